# revision 1
# baseline (speedup 1.0000x reference)
"""MoE all-to-all token dispatch kernel for 8 Trainium2 NeuronCores.

Problem: out[d, t*K+k, :] = x[t, :] if expert_mapping[expert_indices[t, k]] == d
else 0, with B=4, S=4096, H=512, K=2, 64 experts, 8 devices.

Strategy: the output's leading device axis is sharded across the 8 cores —
core d produces out[d] = [T*K, H].  Only ~1/8 of each core's output rows are
nonzero (each (t, k) slot is owned by exactly one device), so instead of
writing the dense 64 MiB slab, each core gathers just its owned token rows
from HBM into SBUF (dma_gather) and scatter-adds them into the owned slots of
the output (dma_scatter_add).  The output DRAM buffer is pre-zeroed by the
runtime (run_bass_kernel_spmd zero-fills/donates ExternalOutput buffers), so
untouched rows are already correct.

Routing metadata (which rows each core owns) is computed on the host from
expert_indices/expert_mapping and passed per-core as int16 index tensors.
Per-core counts are padded to a common multiple-of-CH slot count with
all-valid indices: padded gather slots read one of ZPAD zero rows appended to
xin, and padded scatter slots add those zeros to distinct UNOWNED output rows
(same-row pads serialize HBM read-modify-writes and cost ~2x the kernel;
owned-row pads race the real writes).  The instruction stream is fully static
(one NEFF for all 8 cores, no runtime count registers).

Slab sizes vary ~2.7x with this routing, so heavy slabs export 512-row tail
chunks into other cores' spare "hlp" chunk (a second output tensor) to
equalize per-core slot counts; the host stitches exported rows back during
final assembly.

Work is pipelined chunk by chunk with interleaved issue: gathers on SWDGE
queue 0, scatter-adds on queues 1-3 (they carry ~2/3 of the engine work, so
they get 3 of the 4 rings in the per-engine round-robin).
"""

import numpy as np

B, S, H, K = 4, 4096, 512, 2
T = B * S          # 16384 tokens
TK = T * K         # 32768 output rows per device
D = 8              # devices / NeuronCores
E = 64             # experts

ZPAD = 128         # appended all-zero rows in xin (pad-slot gather targets)
ZROW = T           # index of the first zero row
CH = 512           # slots per chunk (multiple of 128)

TRACE = False
LAST_EXEC_NS = None
LAST_RESULTS = None

_CACHE = {}


def _wrap_idxs16(vals: np.ndarray, maxn: int, pad: int) -> np.ndarray:
    """SWDGE wrapped int16 layout: element i at [i % 16, i // 16], `pad`
    tail, replicated across the 8 partition groups (128 partitions)."""
    arr = np.full(maxn, pad, np.int16)
    arr[: len(vals)] = vals.astype(np.int16)
    w = arr.reshape(maxn // 16, 16).T          # [16, maxn/16]
    return np.ascontiguousarray(np.tile(w, (8, 1)))  # [128, maxn/16]


def _build_module(maxn: int, nch_own: int | None = None):
    from contextlib import ExitStack

    import concourse.bacc as bacc
    import concourse.mybir as mybir
    from concourse.library_config import mlp

    assert maxn % CH == 0
    nb = maxn // 128
    nch = maxn // CH
    if nch_own is None:
        nch_own = nch
    nbc = CH // 128        # data columns per chunk
    wc = CH // 16          # wrapped-idx columns per chunk

    nc = bacc.Bacc("TRN2", debug=False, num_swdge_queues=4)
    xin = nc.dram_tensor("xin", [T + ZPAD, H], mybir.dt.float32,
                         kind="ExternalInput")
    sidx = nc.dram_tensor("sidx", [128, maxn // 16], mybir.dt.int16,
                          kind="ExternalInput")
    didx = nc.dram_tensor("didx", [128, maxn // 16], mybir.dt.int16,
                          kind="ExternalInput")
    out = nc.dram_tensor("out", [TK, H], mybir.dt.float32,
                         kind="ExternalOutput")
    hlp = nc.dram_tensor("hlp", [TK, H], mybir.dt.float32,
                         kind="ExternalOutput")
    hlp2 = nc.dram_tensor("hlp2", [TK, H], mybir.dt.float32,
                          kind="ExternalOutput")

    with (
        nc.Block() as block,
        nc.sbuf_tensor("data", [128, nb, H], mybir.dt.float32) as data,
        nc.sbuf_tensor("sidx_sb", [128, maxn // 16], mybir.dt.int16) as sidx_sb,
        nc.sbuf_tensor("didx_sb", [128, maxn // 16], mybir.dt.int16) as didx_sb,
        nc.semaphore("io0") as io0,
        nc.semaphore("io1") as io1,
        nc.semaphore("ssem0") as ssem0,
        nc.semaphore("ssem1") as ssem1,
        nc.semaphore("ssem2") as ssem2,
        ExitStack() as stack,
    ):
        gsems = [stack.enter_context(nc.semaphore(f"g{c}"))  # noqa: ANT232
                 for c in range(nch)]
        LOOKAHEAD = 4

        @block.gpsimd
        def _(gpsimd):
            gpsimd.load_library(mlp)
            gpsimd.dma_start(sidx_sb[:], sidx[:]).then_inc(io0, 16)
            gpsimd.dma_start(didx_sb[:], didx[:]).then_inc(io1, 16)

            def gather(c):
                gpsimd.dma_gather(
                    data[:, c * nbc:(c + 1) * nbc, :], xin[:],
                    sidx_sb[:, c * wc:(c + 1) * wc], CH, CH, H,
                    single_packet=False, queue_num=0,
                ).then_inc(gsems[c], 16)

            # Interleave issue so scatter DGE starts as soon as its chunk's
            # gather lands instead of after every gather has been emitted.
            gpsimd.wait_ge(io0, 16)
            for c in range(min(LOOKAHEAD, nch)):
                gather(c)
            gpsimd.wait_ge(io1, 16)
            ssems = (ssem0, ssem1, ssem2)
            # scatters carry ~2/3 of the engine work: give them 3 of the 4
            # SWDGE rings so per-engine round-robin matches the load
            for c in range(nch):
                gpsimd.wait_ge(gsems[c], 16)
                tgt = out if c < nch_own else (
                    hlp if c == nch_own else hlp2)
                gpsimd.dma_scatter_add(
                    tgt[:], data[:, c * nbc:(c + 1) * nbc, :],
                    didx_sb[:, c * wc:(c + 1) * wc], CH, CH, H,
                    single_packet=False, queue_num=1 + c % 3,
                ).then_inc(ssems[c % 3], 16)
                if c + LOOKAHEAD < nch:
                    gather(c + LOOKAHEAD)
            for q in range(3):
                gpsimd.wait_ge(ssems[q], 16 * ((nch - q + 2) // 3))

    nc.compile()
    return nc


def kernel(input_tensor, expert_indices, expert_mapping):
    global LAST_EXEC_NS, LAST_RESULTS
    from concourse.bass_utils import run_bass_kernel_spmd

    x = np.zeros((T + ZPAD, H), dtype=np.float32)
    x[:T] = np.asarray(input_tensor, dtype=np.float32).reshape(T, H)
    idx = np.asarray(expert_indices, dtype=np.int32).reshape(-1)
    emap = np.asarray(expert_mapping, dtype=np.int32)
    owner = emap[idx]                                  # [T*K], slot r = t*K+k

    dsts = [np.nonzero(owner == d)[0] for d in range(D)]

    # Balance: heavy slabs export 512-row tail chunks into other cores'
    # spare hlp chunks (up to two, in separate tensors so imports from
    # different slabs cannot collide on a row index), minimizing the common
    # per-core slot count.
    max_own = -(-max(len(v) for v in dsts) // CH)
    best = (max_own, 0)
    for cand in range(max(1, -(-(TK // D) // CH) - 2), max_own + 1):
        units = sum(-(-max(0, len(v) - cand * CH) // CH) for v in dsts)
        nh = -(-units // D)
        if nh <= 2 and cand + nh < best[0] + best[1]:
            best = (cand, nh)
    nch_own, nch_hlp = best
    exports = []                       # (src_core, rows)
    kept = []
    for d in range(D):
        n_exp = -(-max(0, len(dsts[d]) - nch_own * CH) // CH)
        kept.append(dsts[d][: len(dsts[d]) - n_exp * CH])
        for e in range(n_exp):
            lo = len(dsts[d]) - (n_exp - e) * CH
            exports.append((d, dsts[d][lo: lo + CH]))
    nch = nch_own + nch_hlp
    maxn = nch * CH
    imap = {}                          # (importer, hlp_k) -> (src_core, rows)
    slots = [(imp, k) for k in range(nch_hlp)
             for imp in sorted(range(D), key=lambda d: len(kept[d]))]
    for (sc, rows), slot in zip(exports, slots):
        imap[slot] = (sc, rows)

    key = (nch_own, nch_hlp)
    if key not in _CACHE:
        _CACHE[key] = _build_module(maxn, nch_own)
    nc = _CACHE[key]

    empty = np.empty(0, np.int64)

    def _section(rows, cap):
        npad = cap - len(rows)
        mask = np.ones(TK, bool)
        mask[rows] = False
        cand_rows = np.nonzero(mask)[0]
        padrows = cand_rows[:: max(1, len(cand_rows) // max(npad, 1))][:npad]
        pk = np.arange(npad)
        s = np.concatenate([rows // K, ZROW + (pk % ZPAD)])
        t = np.concatenate([rows, padrows])
        return s, t

    in_maps = []
    for d in range(D):
        secs = [(kept[d], nch_own * CH)]
        for k in range(nch_hlp):
            secs.append((imap.get((d, k), (0, empty))[1], CH))
        parts = [_section(r, cap) for r, cap in secs]
        srcfull = np.concatenate([p[0] for p in parts])
        dstfull = np.concatenate([p[1] for p in parts])
        in_maps.append({
            "xin": x,
            "sidx": _wrap_idxs16(srcfull, maxn, pad=0),
            "didx": _wrap_idxs16(dstfull, maxn, pad=0),
        })

    res = run_bass_kernel_spmd(nc, in_maps, list(range(D)), trace=TRACE)
    if TRACE:
        LAST_EXEC_NS = res.exec_time_ns
        LAST_RESULTS = res
    outs = [np.array(res.results[d]["out"]) for d in range(D)]
    for (imp, k), (sc, rows) in imap.items():
        outs[sc][rows] = res.results[imp]["hlp" if k == 0 else "hlp2"][rows]
    return np.stack(outs, axis=0)



# revision 4
# speedup vs baseline: 1.1378x; 1.1378x over previous
"""MoE all-to-all token dispatch kernel for 8 Trainium2 NeuronCores.

Problem: out[d, t*K+k, :] = x[t, :] if expert_mapping[expert_indices[t, k]] == d
else 0, with B=4, S=4096, H=512, K=2, 64 experts, 8 devices.

Strategy: the output's leading device axis is sharded across the 8 cores —
core d produces out[d] = [T*K, H].  Only ~1/8 of each core's output rows are
nonzero, so each core gathers just the needed token rows from HBM into SBUF
(extended dma_gather, cheap to issue: ~1.3us per 1024 rows) and scatters them
into the output.  The output DRAM buffer is pre-zeroed by the runtime, so
untouched rows are already correct.

The scatter mixes two mechanisms, balancing GPSIMD issue time against DMA
engine time (measured: scatter_add RMW packets ~190ns/2KB vs ~100ns for pure
ops; indirect1d pure writes cost ~1.45us of GPSIMD per 128 rows):
  - A leading groups of 1024 rows go through dma_scatter_add (cheap issue,
    expensive RMW DMA).
  - The rest go through indirect_dma_start pure writes, 128 rows per
    instruction (expensive issue, cheap DMA).

Load balancing is 128-row granular: all cores run an identical stream of
NCH chunk-units targeting their own `out` tensor.  Slabs larger than
NCH*128 export 128-row chunks into other cores' spare chunk slots; because
output-row ownership is a partition, exported rows never collide with the
host core's own rows, and the host stitches them back (and re-zeroes them on
the hosting core's slab) during final assembly.  Pad slots gather appended
zero rows of xin and write them to per-core-distinct spare output rows.

Index tensors are loaded via the Sync engine's HWDGE so they land while
GPSIMD is stalled on the ~9us ucode library load.
"""

import numpy as np

B, S, H, K = 4, 4096, 512, 2
T = B * S          # 16384 tokens
TK = T * K         # 32768 output rows per device
D = 8              # devices / NeuronCores
E = 64             # experts

ZPAD = 128         # appended all-zero rows in xin (pad-slot gather targets)
ZROW = T           # index of the first zero row
CH = 128           # slots per chunk-unit
GRP = 8            # chunks per gather / scatter_add group (1024 rows)
A_GROUPS = 1       # leading groups scattered via dma_scatter_add

TRACE = False
LAST_EXEC_NS = None
LAST_RESULTS = None

_CACHE = {}


def _wrap_idxs16(vals: np.ndarray) -> np.ndarray:
    """Extended-instruction SWDGE wrapped int16 layout: element i at
    [i % 16, i // 16], replicated across the 8 partition groups."""
    n = len(vals)
    assert n % 16 == 0
    w = vals.astype(np.int16).reshape(n // 16, 16).T      # [16, n/16]
    return np.ascontiguousarray(np.tile(w, (8, 1)))       # [128, n/16]


def _build_module(nch: int, a_groups: int):
    from contextlib import ExitStack

    import concourse.bacc as bacc
    import concourse.bass as bass
    import concourse.mybir as mybir
    from concourse.library_config import mlp

    maxn = nch * CH
    groups = []                       # (first_chunk, n_chunks)
    c = 0
    while c < nch:
        gsz = min(GRP, nch - c)
        groups.append((c, gsz))
        c += gsz
    a_groups = min(a_groups, len(groups))
    n_add = sum(gsz for _, gsz in groups[:a_groups]) * CH
    ni = nch - n_add // CH            # indirect chunks

    nc = bacc.Bacc("TRN2", debug=False, num_swdge_queues=3)
    xin = nc.dram_tensor("xin", [T + ZPAD, H], mybir.dt.float32,
                         kind="ExternalInput")
    sidx = nc.dram_tensor("sidx", [128, maxn // 16], mybir.dt.int16,
                          kind="ExternalInput")
    didx_a = nc.dram_tensor("didx_a", [128, max(n_add // 16, 16)],
                            mybir.dt.int16, kind="ExternalInput")
    didx_i = nc.dram_tensor("didx_i", [128, max(ni, 1)], mybir.dt.int32,
                            kind="ExternalInput")
    out = nc.dram_tensor("out", [TK, H], mybir.dt.float32,
                         kind="ExternalOutput")

    with (
        nc.Block() as block,
        nc.sbuf_tensor("data", [128, nch, H], mybir.dt.float32) as data,
        nc.sbuf_tensor("sidx_sb", [128, maxn // 16], mybir.dt.int16)
        as sidx_sb,
        nc.sbuf_tensor("didx_a_sb", [128, max(n_add // 16, 16)],
                       mybir.dt.int16) as didx_a_sb,
        nc.sbuf_tensor("didx_i_sb", [128, max(ni, 1)], mybir.dt.int32)
        as didx_i_sb,
        nc.semaphore("io0") as io0,
        nc.semaphore("ssem") as ssem,
        ExitStack() as stack,
    ):
        gsems = [stack.enter_context(nc.semaphore(f"g{g}"))  # noqa: ANT232
                 for g in range(len(groups))]
        LOOK = 2

        @block.sync
        def _(sync):
            # HWDGE loads overlap GPSIMD's ucode library load
            sync.dma_start(sidx_sb[:], sidx[:]).then_inc(io0, 16)
            sync.dma_start(didx_a_sb[:], didx_a[:]).then_inc(io0, 16)
            sync.dma_start(didx_i_sb[:], didx_i[:]).then_inc(io0, 16)

        @block.gpsimd
        def _(gpsimd):
            gpsimd.load_library(mlp)

            def gather(g):
                c0, gsz = groups[g]
                gpsimd.dma_gather(
                    data[:, c0:c0 + gsz, :], xin[:],
                    sidx_sb[:, c0 * 8:(c0 + gsz) * 8], gsz * CH, gsz * CH,
                    H, single_packet=False, queue_num=1,
                ).then_inc(gsems[g], 16)

            gpsimd.wait_ge(io0, 48)
            for g in range(min(LOOK, len(groups))):
                gather(g)
            n_sc = 0
            for g, (c0, gsz) in enumerate(groups):
                gpsimd.wait_ge(gsems[g], 16)
                if g < a_groups:
                    gpsimd.dma_scatter_add(
                        out[:], data[:, c0:c0 + gsz, :],
                        didx_a_sb[:, c0 * 8:(c0 + gsz) * 8], gsz * CH,
                        gsz * CH, H, single_packet=False, queue_num=2,
                    ).then_inc(ssem, 16)
                    n_sc += 1
                else:
                    for ch in range(c0, c0 + gsz):
                        j = ch - n_add // CH
                        gpsimd.indirect_dma_start(
                            out=out[:],
                            out_offset=bass.IndirectOffsetOnAxis(
                                ap=didx_i_sb[:, j:j + 1], axis=0),
                            in_=data[:, ch:ch + 1, :].squeeze(1),
                            in_offset=None,
                        ).then_inc(ssem, 16)
                        n_sc += 1
                if g + LOOK < len(groups):
                    gather(g + LOOK)
            gpsimd.wait_ge(ssem, 16 * n_sc)

    nc.compile()
    return nc


def kernel(input_tensor, expert_indices, expert_mapping):
    global LAST_EXEC_NS, LAST_RESULTS
    from concourse.bass_utils import run_bass_kernel_spmd

    x = np.zeros((T + ZPAD, H), dtype=np.float32)
    x[:T] = np.asarray(input_tensor, dtype=np.float32).reshape(T, H)
    idx = np.asarray(expert_indices, dtype=np.int32).reshape(-1)
    emap = np.asarray(expert_mapping, dtype=np.int32)
    owner = emap[idx]                                  # [T*K], slot r = t*K+k

    dsts = [np.nonzero(owner == d)[0] for d in range(D)]
    sizes = [len(v) for v in dsts]

    # Smallest uniform per-core chunk count nch such that every slab's
    # overflow (in 128-row export chunks) fits into other cores' spare
    # chunk slots.
    nch = -(-max(TK // D, max(sizes)) // CH)
    for cand in range(-(-(TK // D) // CH), nch + 1):
        spare = sum(max(0, cand - (-(-min(s, cand * CH) // CH)))
                    for s in sizes)
        exp = sum(-(-max(0, s - cand * CH) // CH) for s in sizes)
        if spare >= exp:
            nch = cand
            break
    maxn = nch * CH

    kept = [dsts[d][: min(sizes[d], maxn)] for d in range(D)]
    exports = []                       # (owner, rows) in 128-row chunks
    for d in range(D):
        rest = dsts[d][maxn:]
        for lo in range(0, len(rest), CH):
            exports.append((d, rest[lo: lo + CH]))

    # Assign export chunks to cores with spare chunk slots.
    spare_of = [nch - (-(-len(kept[d]) // CH)) for d in range(D)]
    hosted = [[] for _ in range(D)]    # per host core: list of (owner, rows)
    order = sorted(range(D), key=lambda d: -spare_of[d])
    hi = 0
    for exp in exports:
        while spare_of[order[hi % D]] - len(hosted[order[hi % D]]) <= 0:
            hi += 1
        hosted[order[hi % D]].append(exp)
        hi += 1

    key = (nch, A_GROUPS)
    if key not in _CACHE:
        _CACHE[key] = _build_module(nch, A_GROUPS)
    nc = _CACHE[key]

    n_add = min(A_GROUPS * GRP, nch) * CH
    ni = nch - n_add // CH

    in_maps = []
    for d in range(D):
        forbid = np.zeros(TK, bool)
        forbid[kept[d]] = True
        for o, rows in hosted[d]:
            forbid[rows] = True
        free_rows = np.nonzero(~forbid)[0]
        fpos = 0

        # slot sequence: own rows (tail-padded to a chunk boundary), then
        # each hosted export chunk (padded), then all-pad chunks.
        seq_s, seq_t = [], []
        seq_s.append(kept[d] // K)
        seq_t.append(kept[d])
        total = len(kept[d])
        if total % CH:
            npad_c = CH - total % CH
            seq_s.append(ZROW + (np.arange(npad_c) % ZPAD))
            seq_t.append(free_rows[fpos:fpos + npad_c])
            fpos += npad_c
            total += npad_c
        for o, rows in hosted[d]:
            seq_s.append(rows // K)
            seq_t.append(rows)
            total += len(rows)
            if len(rows) % CH:
                npad_c = CH - len(rows) % CH
                seq_s.append(ZROW + (np.arange(npad_c) % ZPAD))
                seq_t.append(free_rows[fpos:fpos + npad_c])
                fpos += npad_c
                total += npad_c
        if total < maxn:
            nrest = maxn - total
            seq_s.append(ZROW + (np.arange(nrest) % ZPAD))
            seq_t.append(free_rows[fpos:fpos + nrest])
            fpos += nrest
        srcfull = np.concatenate(seq_s)
        dstfull = np.concatenate(seq_t)
        assert len(srcfull) == maxn

        didx_i = dstfull[n_add:].astype(np.int32).reshape(ni, CH).T
        in_maps.append({
            "xin": x,
            "sidx": _wrap_idxs16(srcfull),
            "didx_a": _wrap_idxs16(dstfull[:n_add]) if n_add else
            np.zeros((128, 16), np.int16),
            "didx_i": np.ascontiguousarray(didx_i) if ni else
            np.zeros((128, 1), np.int32),
        })

    res = run_bass_kernel_spmd(nc, in_maps, list(range(D)), trace=TRACE)
    if TRACE:
        LAST_EXEC_NS = res.exec_time_ns
        LAST_RESULTS = res
    outs = [np.array(res.results[d]["out"]) for d in range(D)]
    for c in range(D):
        for o, rows in hosted[c]:
            outs[o][rows] = res.results[c]["out"][rows]
            outs[c][rows] = 0.0
    return np.stack(outs, axis=0)
